# revision 3
# baseline (speedup 1.0000x reference)
"""Multi-head attention on 8 TRN2 NeuronCores (data/head-parallel).

Problem: B=4 H=16 S=2048 D=64 fp32 attention, out = softmax(Q K^T / sqrt(D)) V.
B*H = 64 (batch, head) pairs are sharded 8-per-core; each core runs the same
NEFF over its own 8 heads, no collectives.

The baseline bf16 kernel was jointly PE-bound (~260us busy) and ACT-bound
(~261us busy: 33.5M exp at 128 lanes/cycle). This version attacks both:

  - Q K^T runs as fp8 DoubleRow matmuls (0.5 PE cycles per output column,
    2x the bf16 rate). Plain e4m3 is too coarse (3.8% err), so scores are
    computed with a 3-term hi/lo decomposition packed into the DoubleRow
    contraction (128 partitions x 2 rows = 256 slots, d=64 needs 192):
      p<64,i=0: Khi*Qhi   p<64,i=1: (Khi/16)*(16*Qlo)   p>=64,i=0: (16*Klo)*(Qhi/16)
    which yields Qhi*Khi + Qlo*Khi + Qhi*Klo -- only the O(2^-8) QloKlo term
    is dropped. Measured end-to-end rel err 0.0022 (better than bf16's 0.003).
  - exp is split across engines: 9/16 k-tiles on ACT (hw exp), 7/16 on the
    otherwise-idle DVE as a Schraudolph bit-hack: E = bitcast_bf16(int16(
    A*s + B)), one tensor_scalar per tile. Multiplicative sawtooth error
    ~1.8% rms on those tiles; softmax renormalization cancels the mean.
  - P@V stays bf16 (fp8 probabilities/values fail the 2e-2 error budget).
  - The output [d,q]->[q,d] transpose moved off the PE onto the DMA XBAR
    (dma_start_transpose of the bf16 [80,1024] tile), freeing PSUM so both
    score and output accumulators double-buffer (4+4 banks).
  - Software pipeline: iteration g runs QK+exp of chunk g interleaved with
    P@V of chunk g-1 on the PE, then the PSUM->SBUF copy of g-1 (ACT) and
    the normalize+store of g-2 (DVE reciprocal+scale, Sync DMA).
"""

import math
from contextlib import ExitStack

import ml_dtypes
import numpy as np

import concourse.bass as bass
import concourse.bacc as bacc
import concourse.tile as tile
import concourse.mybir as mybir
from concourse.bass_utils import run_bass_kernel_spmd

B, H, S, D = 4, 16, 2048, 64
N_CORES = 8
HPC = B * H // N_CORES     # heads per core
ST = S // 128              # 16 k-tiles of 128
QCHUNK = 1024              # q processed in chunks (PSUM budget)
NQ = S // QCHUNK
NDR = QCHUNK // 256        # DoubleRow matmuls per (k-tile, chunk), N=256 each
NJ = QCHUNK // 128         # 128-q output groups per chunk
DT = mybir.dt

# k-tiles whose exp runs on DVE (Schraudolph); the rest go to ACT.
DVE_TILES = frozenset({1, 3, 5, 7, 9, 11, 13})
# Schraudolph int16/bf16 exp: E = bitcast_bf16(int16(A*s + B)), trunc-calibrated
SCHRAUD_A = 128.0 / math.log(2.0)                # * scale at runtime
SCHRAUD_B = 127.0 * 128.0 + 0.5 - 0.0430 * 128.0

_BUILT = {}


class _Bacc(bacc.Bacc):
    """Bacc with the move-matmul-waits-to-ldweights pass disabled: keeping
    waits on the matmul (not its LDWEIGHTS) lets the PE queue pull weight
    loads ahead of in-flight matmuls, hiding the LDW cost."""

    def move_matmul_waits_to_ldweights(self):
        pass


def _load_head(nc, stage, qt_d, kt_d, vp_d, h, first):
    qt = stage.tile([128, 2, S], DT.float8e4, tag="qt")
    kt = stage.tile([128, 2, S], DT.float8e4, tag="kt")
    vp = stage.tile([128, ST, 128], DT.bfloat16, tag="vp")
    for j in range(2):
        half = slice(j * (S // 2), (j + 1) * (S // 2))
        # Cold start: head 0's first halves ride the idle Sync/Scalar HWDGEs.
        keng = nc.sync if (first and j == 0) else nc.gpsimd
        qeng = nc.scalar if (first and j == 0) else nc.gpsimd
        keng.dma_start(out=kt[:, :, half], in_=kt_d[h][:, :, half])
        qeng.dma_start(out=qt[:, :, half], in_=qt_d[h][:, :, half])
    vp_v = vp_d[h].rearrange("(t p) e -> p t e", p=128)
    for j in range(2):
        sl = slice(8 * j, 8 * j + 8)
        nc.gpsimd.dma_start(out=vp[:, sl, :], in_=vp_v[:, sl, :])
    return qt, kt, vp


def build_graph(scale: float, heads: int = HPC):
    nc = _Bacc("TRN2", target_bir_lowering=False, debug=False,
               num_devices=N_CORES)
    qt_d = nc.dram_tensor("QT8", [heads, 128, 2, S], DT.float8e4,
                          kind="ExternalInput").ap()
    kt_d = nc.dram_tensor("KT8", [heads, 128, 2, S], DT.float8e4,
                          kind="ExternalInput").ap()
    vp_d = nc.dram_tensor("VP", [heads, S, 128], DT.bfloat16,
                          kind="ExternalInput").ap()
    o_d = nc.dram_tensor("out", [heads, S, D], DT.float32,
                         kind="ExternalOutput").ap()

    a_s = float(scale) * SCHRAUD_A

    with tile.TileContext(nc) as tc, ExitStack() as ctx:
        stage = ctx.enter_context(tc.tile_pool(name="stage", bufs=3))
        epool = ctx.enter_context(tc.tile_pool(name="epool", bufs=2))
        spool = ctx.enter_context(tc.tile_pool(name="spool", bufs=2))
        trp = ctx.enter_context(tc.tile_pool(name="trp", bufs=2))
        outp = ctx.enter_context(tc.tile_pool(name="outp", bufs=2))
        recp = ctx.enter_context(tc.tile_pool(name="recp", bufs=2))
        ps_st = ctx.enter_context(tc.tile_pool(name="ps_st", bufs=2, space="PSUM"))
        ps_ot = ctx.enter_context(tc.tile_pool(name="ps_ot", bufs=2, space="PSUM"))

        gs = [(h, c) for h in range(heads) for c in range(NQ)]
        head_tiles = {}
        state = {}   # g -> dict(ets, vp, ot, ots, otr, h, c)

        head_tiles[0] = _load_head(nc, stage, qt_d, kt_d, vp_d, 0, True)

        for i in range(len(gs) + 2):
            cur = gs[i] if i < len(gs) else None
            if cur is not None:
                h, c = cur
                if c == NQ - 1 and h + 1 < heads:
                    head_tiles[h + 1] = _load_head(nc, stage, qt_d, kt_d,
                                                   vp_d, h + 1, False)
                qt, kt, vp = head_tiles[h]
                q0 = c * QCHUNK
                st_tiles = []
                ets = []
                state[i] = {"h": h, "c": c, "vp": vp, "ets": ets}
            prev = state.get(i - 1)
            fin = state.pop(i - 2, None)

            for t in range(ST):
                if cur is not None:
                    st = ps_st.tile([128, QCHUNK], DT.float32, tag="st")
                    for n in range(NDR):
                        nc.tensor.matmul(
                            st[:, n * 256:(n + 1) * 256],
                            lhsT=kt[:, :, t * 128:(t + 1) * 128],
                            rhs=qt[:, :, q0 + n * 256:q0 + (n + 1) * 256],
                            start=True, stop=True,
                            perf_mode=mybir.MatmulPerfMode.DoubleRow,
                        )
                    if t in DVE_TILES:
                        eti = epool.tile([128, QCHUNK], DT.int16, tag=f"et{t}")
                        nc.vector.tensor_scalar(
                            eti, st, a_s, SCHRAUD_B,
                            mybir.AluOpType.mult, mybir.AluOpType.add)
                        ets.append(eti.bitcast(DT.bfloat16))
                    else:
                        et = epool.tile([128, QCHUNK], DT.bfloat16, tag=f"et{t}")
                        nc.scalar.activation(
                            out=et, in_=st,
                            func=mybir.ActivationFunctionType.Exp, scale=scale)
                        ets.append(et)
                if prev is not None:
                    if t == 0:
                        prev["ot"] = ps_ot.tile([128, QCHUNK], DT.float32,
                                                tag="ot", name="ot")
                    for n in range(2):
                        nc.tensor.matmul(
                            prev["ot"][:, n * 512:(n + 1) * 512],
                            lhsT=prev["vp"][:, t, :],
                            rhs=prev["ets"][t][:, n * 512:(n + 1) * 512],
                            start=(t == 0), stop=(t == ST - 1),
                        )

            if prev is not None:
                # PSUM -> SBUF as bf16 (ACT), then [80,1024] -> [1024,80] on
                # the DMA XBAR.  Rows 65..79 are the zero-padded V columns.
                ots = spool.tile([80, QCHUNK], DT.bfloat16, tag="ots")
                nc.scalar.copy(out=ots, in_=prev["ot"][0:80, :])
                otr = trp.tile([128, NJ, 80], DT.bfloat16, tag="otr")
                nc.sync.dma_start_transpose(out=otr, in_=ots)
                prev["otr"] = otr

            if fin is not None:
                otr = fin["otr"]
                rec = recp.tile([128, NJ], DT.float32, tag="rec")
                nc.vector.reciprocal(out=rec, in_=otr[:, :, D])
                outst = outp.tile([128, NJ, D], DT.float32, tag="outst")
                for j in range(NJ):
                    nc.vector.tensor_scalar(
                        outst[:, j, :], otr[:, j, 0:D], rec[:, j:j + 1],
                        None, mybir.AluOpType.mult)
                o_v = o_d[fin["h"], fin["c"] * QCHUNK:(fin["c"] + 1) * QCHUNK, :]
                o_v = o_v.rearrange("(r p) d -> p r d", p=128)
                nc.sync.dma_start(out=o_v, in_=outst)

    nc.compile()
    return nc


def _get_nc(scale: float):
    key = round(float(scale), 9)
    if key not in _BUILT:
        _BUILT[key] = build_graph(float(scale))
    return _BUILT[key]


def shard_inputs(Q, K, V):
    """Host-side prep: shard heads across cores; Q/K transposed to [D,S] and
    split into the 3-term hi/lo e4m3 DoubleRow layout; V gets a ones column
    (softmax denominators fall out of the P@V matmul) and bf16."""
    bf16 = ml_dtypes.bfloat16
    e4m3 = ml_dtypes.float8_e4m3
    BH = B * H
    qs = np.asarray(Q, dtype=np.float32).reshape(BH, S, D).transpose(0, 2, 1)
    ks = np.asarray(K, dtype=np.float32).reshape(BH, S, D).transpose(0, 2, 1)
    vs = np.asarray(V, dtype=np.float32).reshape(BH, S, D)

    def hi_lo(x):
        hi = x.astype(e4m3)
        hif = hi.astype(np.float32)
        lo16 = ((x - hif) * np.float32(16.0)).astype(e4m3)
        hi16 = (hif / np.float32(16.0)).astype(e4m3)
        return hi, lo16, hi16

    qhi, qlo16, qhi16 = hi_lo(qs)
    khi, klo16, khi16 = hi_lo(ks)

    qt = np.zeros((BH, 128, 2, S), dtype=e4m3)
    kt = np.zeros((BH, 128, 2, S), dtype=e4m3)
    qt[:, :D, 0, :] = qhi
    qt[:, :D, 1, :] = qlo16
    qt[:, D:, 0, :] = qhi16
    kt[:, :D, 0, :] = khi
    kt[:, :D, 1, :] = khi16
    kt[:, D:, 0, :] = klo16

    vp = np.zeros((BH, S, 128), dtype=bf16)
    vp[:, :, :D] = vs.astype(bf16)
    vp[:, :, D] = np.float32(1.0)

    in_maps = []
    for c in range(N_CORES):
        sl = slice(c * HPC, (c + 1) * HPC)
        in_maps.append({
            "QT8": np.ascontiguousarray(qt[sl]),
            "KT8": np.ascontiguousarray(kt[sl]),
            "VP": np.ascontiguousarray(vp[sl]),
        })
    return in_maps


def kernel(Q, K, V, d_k, **run_kwargs):
    scale = 1.0 / math.sqrt(float(d_k))
    nc = _get_nc(scale)
    in_maps = shard_inputs(Q, K, V)
    res = run_bass_kernel_spmd(nc, in_maps, core_ids=list(range(N_CORES)),
                               **run_kwargs)
    out = np.concatenate([r["out"] for r in res.results], axis=0)
    out = out.reshape(B, H, S, D).astype(np.float32)
    kernel.last_results = res
    return out
